# revision 1
# baseline (speedup 1.0000x reference)
"""BERT self-attention (B=2, S=2048, D=768, H=12, DH=64) on 8 trn2 NeuronCores.

Sharding: data parallel on batch x tensor parallel on heads. Core c handles
batch b = c // 4 and heads h0..h0+2 with h0 = 3 * (c % 4) — 24 (b, h) units,
3 per core.

Per-core kernel (all layouts chosen so nothing is transposed on-chip):
  - hidden^T [768, 2048] arrives k-major; W^T slices arrive as stationary
    groups. Q^T/K^T [64, 2048] come straight out of the projection matmuls
    (head dim on partitions); V comes out token-major [2048, 64] by swapping
    stationary/moving operands. Biases are folded in as one extra K=1
    accumulating matmul per output tile (bias x ones-row rank-1 update).
  - Scores are computed transposed: S^T[j, i] = K^T.T @ Q^T per 128-key block,
    so the softmax probs are already key-major for the P @ V contraction.
  - exp runs on ScalarE straight out of PSUM with the 1/sqrt(DH) scale and the
    additive attention mask fused into the activation's scale/bias. No max
    subtraction: scores here are ~N(0, 1) + mask, far from fp32 exp overflow.
  - V's stationary operand is padded to 128 columns with ones, so the P @ V
    matmul emits ctx^T on psum rows 0:64 and 64 broadcast copies of the
    softmax denominator on rows 64:128 (full-width FWL weight loads and a
    free denominator broadcast). Normalize = approx-reciprocal + multiply on
    VectorE while draining PSUM.
  - All matmul operands are fp16 (PSUM accumulation stays fp32): fp32
    operands stream at 1/2-1/4 rate through the PE; fp16 runs at full rate
    with ~1e-3 scale-relative output error vs the fp32 reference.
  - Emission order is hand-interleaved round-by-round (scores/exp paced by
    ScalarE with V, later heads' projections, and the previous round's P @ V
    woven between score pairs) because each engine executes its instruction
    stream in order.
Output per core is head-major transposed [3, 64, 2048]; the host assembles the
full [B, S, D] tensor (pure unsharding/layout, no arithmetic).
"""

import numpy as np

import concourse.bass as bass
import concourse.mybir as mybir
import concourse.tile as tile
from concourse import bacc
from concourse.bass import ts, ds
from concourse.bass_utils import run_bass_kernel_spmd

B, S, D = 2, 2048, 768
H, DH = 12, 64
NH = 3            # heads per core
N_CORES = 8
KC = D // 128     # contraction chunks (6)
NJ = S // 128     # key blocks (16)
IB = 1024         # query block (i) processed per exp/PV round
MM_DT = mybir.dt.float16      # matmul operand dtype (psum accum stays f32)
TRACE = False     # set True (from test.py) to capture an NTFF profile
LAST_RESULT = {}  # exec_time_ns etc. for test.py

f32 = mybir.dt.float32
f16 = mybir.dt.float16
AF = mybir.ActivationFunctionType

_NC_CACHE = None


def build_nc():
    nc = bacc.Bacc("TRN2", target_bir_lowering=False, debug=False,
                   num_devices=N_CORES)
    hidT_d = nc.dram_tensor("hidT", [128, KC, S], MM_DT, kind="ExternalInput")
    wT_d = nc.dram_tensor("wT", [128, KC, 576], MM_DT, kind="ExternalInput")
    bias_d = nc.dram_tensor("biasrow", [1, 576], MM_DT, kind="ExternalInput")
    bias2_d = nc.dram_tensor("bias2", [128, NH], f32, kind="ExternalInput")
    mask_d = nc.dram_tensor("maskT", [128, NJ], f32, kind="ExternalInput")
    out_d = nc.dram_tensor("out", [NH, DH, S], f32, kind="ExternalOutput")

    with tile.TileContext(nc) as tc:
        with (
            tc.tile_pool(name="const", bufs=1) as cpool,
            tc.tile_pool(name="proj", bufs=1) as proj,
            tc.tile_pool(name="hid", bufs=1) as hpool,
            tc.tile_pool(name="wts", bufs=1) as wpool,
            tc.tile_pool(name="expS", bufs=3) as epool,
            tc.tile_pool(name="psS", bufs=2, space="PSUM") as psS,
            tc.tile_pool(name="psQKV", bufs=2, space="PSUM") as psQKV,
            tc.tile_pool(name="psC", bufs=2, space="PSUM") as psC,
            tc.tile_pool(name="den", bufs=4) as dpool,
            tc.tile_pool(name="rb", bufs=3) as rpool,
            tc.tile_pool(name="ost", bufs=3) as opool,
        ):
            ones = cpool.tile([1, 512], MM_DT)
            nc.vector.memset(ones[:], 1.0)
            biasrow = cpool.tile([1, 576], MM_DT)
            nc.sync.dma_start(biasrow[:], bias_d[:])
            bias2 = cpool.tile([128, NH], f32)
            nc.sync.dma_start(bias2[:], bias2_d[:])
            maskT = cpool.tile([128, NJ], f32)
            nc.sync.dma_start(maskT[:], mask_d[:])

            # qk2 rows 0:64 = Q^T (drained), rows 64:128 = copy of Q^T;
            # k2 rows 64:128 = K^T, rows 0:64 = copy. Score matmuls for
            # even/odd key blocks run on the lower/upper array halves so
            # adjacent j-blocks execute concurrently (row-group tiling).
            qk2 = proj.tile([128, NH, S], MM_DT)
            k2 = proj.tile([128, NH, S], MM_DT)
            # vAug cols 0:64 = V, cols 64:128 stay 1.0: the P@V matmul then
            # emits ctx^T on psum rows 0:64 and 64 broadcast copies of the
            # softmax denominator on rows 64:128 — 128-wide weight loads
            # (FWL) and a free denominator broadcast.
            vAug = proj.tile([128, NH, NJ, 2 * DH], MM_DT)
            nc.vector.memset(vAug[:, :, :, DH:2 * DH], 1.0)

            hidT = hpool.tile([128, KC, S], MM_DT)
            wT = wpool.tile([128, KC, 576], MM_DT)
            # each dma_start costs ~0.6us of serial issue time on the Sync
            # engine — batch the input loads into a few wide transfers and
            # defer the second half of the tokens into round 0
            nc.sync.dma_start(wT[:], wT_d[:])
            nc.sync.dma_start(hidT[:, :, 0:1024], hidT_d[:, :, 0:1024])

            def emit_hid_slice(t):
                nc.sync.dma_start(hidT[:, :, ts(t, 512)],
                                  hidT_d[:, :, ts(t, 512)])

            def emit_qk_t(h, t):
                # stationary = [Wq_h^T | Wk_h^T]; psum rows 0:64 = Q^T,
                # rows 64:128 = K^T; bias folded into the drain.
                ps = psQKV.tile([128, 512], f32, tag="ps")
                for c in range(KC):
                    nc.tensor.matmul(
                        ps[:], wT[:, c, ts(h, 128)], hidT[:, c, ts(t, 512)],
                        start=(c == 0), stop=(c == KC - 1))
                nc.vector.tensor_scalar_add(
                    qk2[0:64, h, ts(t, 512)], ps[0:64, :], bias2[0:64, h:h + 1])
                nc.vector.tensor_scalar_add(
                    k2[64:128, h, ts(t, 512)], ps[64:128, :],
                    bias2[64:128, h:h + 1])
                nc.sync.dma_start(qk2[64:128, h, ts(t, 512)],
                                  qk2[0:64, h, ts(t, 512)])
                nc.sync.dma_start(k2[0:64, h, ts(t, 512)],
                                  k2[64:128, h, ts(t, 512)])

            def emit_v_t(t):
                # V token-major: stationary = hidden^T chunk, moving = Wv^T.
                ps = psQKV.tile([128, 192], f32, tag="ps")
                for c in range(KC):
                    nc.tensor.matmul(
                        ps[:], hidT[:, c, ts(t, 128)], wT[:, c, 384:576],
                        start=(c == 0), stop=False)
                nc.tensor.matmul(  # + ones x bv  (K=1)
                    ps[:], ones[0:1, 0:128], biasrow[0:1, 384:576],
                    start=False, stop=True)
                nc.vector.tensor_copy(
                    vAug[:, :, t, 0:DH],
                    ps[:].rearrange("p (h d) -> p h d", h=NH))

            def emit_s_j(h, ib, eS, j):
                sl = slice(0, 64) if j % 2 == 0 else slice(64, 128)
                ps = psS.tile([128, IB], f32, tag="psS")
                for n in range(IB // 512):
                    nc.tensor.matmul(
                        ps[:, ts(n, 512)], k2[sl, h, ts(j, 128)],
                        qk2[sl, h, ds(ib * IB + n * 512, 512)],
                        start=True, stop=True)
                nc.scalar.activation(eS[:, j, :], ps[:], AF.Exp,
                                     bias=maskT[:, j:j + 1], scale=0.125)

            def emit_pv_j(h, pcs, eS, j):
                for it in range(IB // 512):
                    nc.tensor.matmul(
                        pcs[it][:], vAug[:, h, j, :], eS[:, j, ts(it, 512)],
                        start=(j == 0), stop=(j == NJ - 1))

            def emit_norm(h, ib, pcs):
                for it in range(IB // 512):
                    pc = pcs[it]
                    # rows 64:128 of pc are 64 copies of the denominator
                    dB = dpool.tile([128, 512], f32, tag="dn")
                    nc.vector.tensor_copy(dB[64:128, :], pc[64:128, :])
                    dLo = dpool.tile([64, 512], f32, tag="dlo")
                    nc.sync.dma_start(dLo[:], dB[64:128, :])
                    rB = rpool.tile([64, 512], f32, tag="rb")
                    nc.vector.reciprocal_approx_fast(rB[:], dLo[:])
                    o = opool.tile([64, 512], f32, tag="ost")
                    nc.vector.tensor_mul(o[:], pc[0:DH, :], rB[:])
                    nc.sync.dma_start(
                        out_d[h, :, ds(ib * IB + it * 512, 512)], o[:])

            # Round-interleaved emission: per-engine instruction order is
            # the schedule, so the exp-paced score loop is the backbone and
            # everything else (V, later heads' QK, previous round's P@V) is
            # woven between score pairs to keep ScalarE (the bottleneck
            # engine) continuously fed.
            rounds = [(h, ib) for h in range(NH) for ib in range(S // IB)]
            emit_qk_t(0, 0)
            emit_qk_t(0, 1)
            prev = None           # (h, ib, eS) of previous round
            mypcs = None
            for r, (h, ib) in enumerate(rounds):
                is_last = (r == len(rounds) - 1)
                eS = epool.tile([128, NJ, IB], MM_DT, tag="eS")
                pcs = None
                if prev is not None:
                    pcs = [psC.tile([128, 512], f32, tag="psC",
                                    name=f"pc_{r}_{it}")
                           for it in range(IB // 512)]
                for j in range(NJ):
                    if r == 0 and j in (2, 5):    # deferred hidden slices
                        emit_hid_slice(2 + (j == 5))
                    if r == 0 and j in (8, 12):   # rest of head-0 proj
                        emit_qk_t(0, j // 4)
                    emit_s_j(h, ib, eS, j)
                    if prev is not None:
                        # drain the previous round's P@V at double rate so
                        # its psum frees mid-round (norm at j == 8)
                        if j < NJ // 2:
                            emit_pv_j(prev[0], pcs, prev[2], 2 * j)
                            emit_pv_j(prev[0], pcs, prev[2], 2 * j + 1)
                        elif j == NJ // 2:
                            emit_norm(prev[0], prev[1], pcs)
                    if is_last and j >= NJ // 2:
                        # last round: P@V chases its own exps inline
                        if j == NJ // 2:
                            mypcs = [psC.tile([128, 512], f32, tag="psC",
                                              name=f"pc_last_{it}")
                                     for it in range(IB // 512)]
                        emit_pv_j(h, mypcs, eS, 2 * (j - NJ // 2))
                        emit_pv_j(h, mypcs, eS, 2 * (j - NJ // 2) + 1)
                    if r == 0:
                        emit_v_t(j)           # V for all heads, one t per j
                    elif r == 1 and j % 4 == 0:
                        emit_qk_t(1, j // 4)  # head 1 projections
                    elif r == 2 and j % 4 == 0:
                        emit_qk_t(2, j // 4)  # head 2 projections
                prev = (h, ib, eS)
            emit_norm(prev[0], prev[1], mypcs)
    nc.compile()
    return nc


def _prep_core_inputs(c, hidden_states, attention_mask, Wq, bq, Wk, bk, Wv, bv):
    b, h0 = c // 4, NH * (c % 4)
    rows = slice(h0 * DH, (h0 + NH) * DH)
    Wq_s, Wk_s, Wv_s = Wq[rows], Wk[rows], Wv[rows]      # [192, 768] each
    groups = []
    for h in range(NH):
        groups.append(Wq_s[h * DH:(h + 1) * DH])
        groups.append(Wk_s[h * DH:(h + 1) * DH])
    groups.append(Wv_s)
    big = np.concatenate(groups, axis=0)                 # [576, 768]
    wT = np.ascontiguousarray(
        big.T.reshape(KC, 128, 576).transpose(1, 0, 2)).astype(np.float16)
    hidT = np.ascontiguousarray(
        hidden_states[b].T.reshape(KC, 128, S).transpose(1, 0, 2)).astype(np.float16)
    bias_groups = []
    for h in range(NH):
        bias_groups.append(bq[rows][h * DH:(h + 1) * DH])
        bias_groups.append(bk[rows][h * DH:(h + 1) * DH])
    bias_groups.append(bv[rows])
    biasrow = np.concatenate(bias_groups)[None, :].astype(np.float16)
    cols = []
    for h in range(NH):
        cols.append(np.concatenate([bq[rows][h * DH:(h + 1) * DH],
                                    bk[rows][h * DH:(h + 1) * DH]]))
    bias2 = np.stack(cols, axis=1).astype(np.float32)    # [128, NH]
    maskT = np.ascontiguousarray(
        attention_mask[b, 0, 0].reshape(NJ, 128).T)      # [128, NJ]
    return {"hidT": hidT, "wT": wT, "biasrow": biasrow, "bias2": bias2,
            "maskT": maskT}


def kernel(hidden_states, attention_mask, Wq, bq, Wk, bk, Wv, bv):
    global _NC_CACHE, LAST_RESULT
    hidden_states = np.asarray(hidden_states, dtype=np.float32)
    attention_mask = np.asarray(attention_mask, dtype=np.float32)
    if _NC_CACHE is None:
        _NC_CACHE = build_nc()
    nc = _NC_CACHE
    in_maps = [
        _prep_core_inputs(c, hidden_states, attention_mask,
                          np.asarray(Wq), np.asarray(bq), np.asarray(Wk),
                          np.asarray(bk), np.asarray(Wv), np.asarray(bv))
        for c in range(N_CORES)
    ]
    res = run_bass_kernel_spmd(nc, in_maps, core_ids=list(range(N_CORES)),
                               trace=TRACE)
    LAST_RESULT = {"exec_time_ns": res.exec_time_ns,
                   "trace": res.instructions_and_trace}
    out = np.empty((B, S, H * DH), dtype=np.float32)
    for c in range(N_CORES):
        b, h0 = c // 4, NH * (c % 4)
        r = res.results[c]["out"]                        # [NH, DH, S]
        out[b, :, h0 * DH:(h0 + NH) * DH] = r.reshape(NH * DH, S).T
    return out

